# revision 1
# baseline (speedup 1.0000x reference)
"""DistMaps Trainium2 kernel (nn_DistMaps_28217935135424).

kernel(x, coords) -> [8, 2, 512, 512] float32
  out[b, m, h, w] = tanh(2*sqrt(min_i d_i)),
  d_i = ((h-r_i)/5)^2 + ((w-c_i)/5)^2 for the 12 points of (batch b, half m),
  invalid points (max(r,c) < 0) contribute 1e6.

Sharding: data-parallel over batch B=8 across the 8 NeuronCores (one batch
per core). x is only used for its shape; the output depends on coords alone,
so only coords (plus small compile-time constants) are sent to the device.

Device algorithm (per core): tanh(2*sqrt(d)) saturates to 1.0f beyond ~21.7px
from any point, so only a 48-wide column window crossing two 128-row tiles
around each point can differ from 1.0. The d map [128, 4*512] is memset to
1e6; per point a [128, 2x48] strip is computed on the PE (K=1 ones x CLUT
window broadcast + K=4 selector matmul adding the row part) and min-combined
into the d map by one vector op at a runtime offset (registers loaded from
SBUF). Then sqrt and tanh passes (ACT LUTs) and DMA out.
"""
from contextlib import ExitStack

import numpy as np

import concourse.bass as bass
import concourse.tile as tile
from concourse import bacc, mybir
from concourse.tile_rust import add_dep_helper
from concourse.bass_utils import run_bass_kernel_spmd

F32 = mybir.dt.float32
I32 = mybir.dt.int32
AF = mybir.ActivationFunctionType
OP = mybir.AluOpType

B = 8
H = W = 512
NPTS = 24
NPM = 12
NT = 4
WS = 48
CMARG = WS // 2
TS2 = 2 * WS
UNIT = 256
BIG = 1.0e6
CBW = W + 512 + 1


def _make_consts():
    x = np.arange(W, dtype=np.float32)
    u = np.arange(512)
    t = u // 128
    uu = u % 128
    c = uu // WS
    tsel = np.where(c <= 1, t - c, -99.0).astype(np.float32)
    cb = np.zeros((NPTS, CBW), dtype=np.float32)
    cb[:, 0:W] = x[None, :]
    cb[:, W:W + 512] = tsel[None, :]
    cb[:, CBW - 1] = np.arange(NPTS, dtype=np.float32)
    return {"cb": cb}


def _build():
    nc = bacc.Bacc("TRN2", target_bir_lowering=False, debug=False, num_devices=B)
    coords = nc.dram_tensor("coords", [NPTS, 3], F32, kind="ExternalInput").ap()
    cbin = nc.dram_tensor("cb", [NPTS, CBW], F32, kind="ExternalInput").ap()
    y = nc.dram_tensor("y", [2, H, W], F32, kind="ExternalOutput").ap()

    with tile.TileContext(nc) as tc, ExitStack() as ctx:
        pool = ctx.enter_context(tc.tile_pool(name="sb", bufs=1))
        d_pool = ctx.enter_context(tc.tile_pool(name="dmap", bufs=1))
        psum_cl = ctx.enter_context(tc.tile_pool(name="psum_cl", bufs=6, space="PSUM"))

        coords_sb = pool.tile([NPTS, 3], F32, tag="coords_sb")
        nc.sync.dma_start(coords_sb[:], coords[:])
        cb = pool.tile([NPTS, CBW], F32, tag="cb_sb")
        nc.scalar.dma_start(cb[:], cbin[:])
        gw = cb[:, 0:W]
        tselc = cb[:, W:W + 512]
        pidx = cb[:, CBW - 1:CBW]

        ones_sb = pool.tile([1, 128], F32, tag="ones_sb")
        nc.gpsimd.memset(ones_sb[:], 1.0)

        r = coords_sb[:, 0:1]
        c = coords_sb[:, 1:2]

        pen = pool.tile([NPTS, 1], F32, tag="pen")
        nc.vector.tensor_tensor(out=pen[:], in0=r, in1=c, op=OP.max)
        nc.vector.tensor_scalar(out=pen[:], in0=pen[:], scalar1=0.0,
                                scalar2=BIG, op0=OP.is_lt, op1=OP.mult)

        # t0 = floor(clip((r-23)/128, 0, 2)) as (y>=1)+(y>=2): exact under any
        # f32->i32 cast rounding mode (HW rounds to nearest, sim truncates).
        geo = pool.tile([NPTS, 2], F32, tag="geo")
        yrow = pool.tile([NPTS, 1], F32, tag="yrow")
        nc.vector.tensor_scalar(out=yrow[:], in0=r, scalar1=-23.0,
                                scalar2=1.0 / 128.0, op0=OP.add, op1=OP.mult)
        nc.vector.tensor_scalar(out=geo[:, 0:1], in0=yrow[:], scalar1=1.0,
                                scalar2=None, op0=OP.is_ge)
        nc.vector.scalar_tensor_tensor(out=geo[:, 0:1], in0=yrow[:], scalar=2.0,
                                       in1=geo[:, 0:1], op0=OP.is_ge, op1=OP.add)
        nc.vector.tensor_scalar(out=geo[:, 1:2], in0=c, scalar1=float(-CMARG),
                                scalar2=0.0, op0=OP.add, op1=OP.max)
        nc.vector.tensor_scalar(out=geo[:, 1:2], in0=geo[:, 1:2],
                                scalar1=float(W - WS), scalar2=None, op0=OP.min)
        gint = pool.tile([NPTS, 2], I32, tag="gint")  # t0, cs
        nc.vector.tensor_copy(gint[:], geo[:])
        t0ff = pool.tile([NPTS, 1], F32, tag="t0ff")
        nc.vector.tensor_copy(t0ff[:], gint[:, 0:1])
        # woff = 512*i + cs; round(512i + cs_f) == 512i + round(cs_f) for both
        # rounding modes (512i even), so it stays consistent with gint cs.
        woff_f = pool.tile([NPTS, 1], F32, tag="woff_f")
        nc.vector.tensor_scalar(out=woff_f[:], in0=pidx, scalar1=float(W),
                                scalar2=None, op0=OP.mult)
        nc.vector.tensor_tensor(out=woff_f[:], in0=woff_f[:], in1=geo[:, 1:2],
                                op=OP.add)
        giw = pool.tile([NPTS, 1], I32, tag="giw")
        nc.vector.tensor_copy(giw[:], woff_f[:])

        # CLUT[i, w] = ((w - c_i)/5)^2, flattened to one partition for matmul rhs
        cdiff = pool.tile([NPTS, W], F32, tag="cdiff")
        nc.vector.tensor_scalar(out=cdiff[:], in0=gw, scalar1=c,
                                scalar2=None, op0=OP.subtract)
        clut = pool.tile([NPTS, W], F32, tag="clut")
        nc.scalar.activation(clut[:], cdiff[:], AF.Square, scale=0.2)
        clut1 = pool.tile([1, NPTS * W], F32, tag="clut1")
        nc.sync.dma_start(clut1[:].rearrange("o (i w) -> o i w", i=NPTS), clut[:])

        # RS [24, 1024]: rowpart ((h-r_i)/5)^2 + pen | tile-selector (padded)
        rs = pool.tile([NPTS, 1024], F32, tag="rs")
        rdiff = pool.tile([NPTS, W], F32, tag="rdiff")
        nc.vector.tensor_scalar(out=rdiff[:], in0=gw, scalar1=r,
                                scalar2=None, op0=OP.subtract)
        nc.scalar.activation(rs[:, 0:512], rdiff[:], AF.Square, scale=0.2)
        nc.vector.tensor_scalar(out=rs[:, 0:512], in0=rs[:, 0:512], scalar1=pen[:],
                                scalar2=None, op0=OP.add)
        nc.vector.tensor_scalar(out=rs[:, 512:1024], in0=tselc, scalar1=t0ff[:],
                                scalar2=None, op0=OP.is_equal)

        # rsf[t, 256i + (0:128 rowpart | 128:224 sel | pad)]
        rsf = pool.tile([NT, NPTS * UNIT], F32, tag="rsf")
        rs_v = rs[:].rearrange("i (s x) -> i s x", s=2)
        flat_engs = [nc.sync, nc.scalar, nc.gpsimd, nc.sync]
        for t in range(NT):
            flat_engs[t].dma_start(
                rsf[t:t + 1, :].rearrange("o (i s p) -> o i s p", i=NPTS, s=2),
                rs_v[:, :, 128 * t:128 * (t + 1)])

        dmaps = []
        for m in range(2):
            d = d_pool.tile([128, NT * W], F32, tag=f"d{m}")
            nc.gpsimd.memset(d[:], BIG)
            dmaps.append(d)

        for m in range(2):
            d4 = dmaps[m][:].rearrange("p (t w) -> p t w", t=NT)
            for k in range(NPM):
                i = m * NPM + k
                clb = psum_cl.tile([128, TS2], F32)
                with nc.tensor.register() as rw:
                    nc.tensor.reg_load(rw, giw[i:i + 1, 0:1])
                    woff = bass.make_scalar_value(rw, min_val=0,
                                                  max_val=NPTS * W - WS)
                    rhs = clut1[0:1, bass.ds(woff, WS)]
                    rhs2 = rhs.rearrange("p w -> p () w").to_broadcast((1, 2, WS))
                    nc.tensor.matmul(clb[:].rearrange("p (c w) -> p c w", c=2),
                                     ones_sb[:], rhs2, start=True, stop=True)
                nc.tensor.matmul(clb[:], rsf[:, UNIT * i:UNIT * i + 128],
                                 rsf[:, UNIT * i + 128:UNIT * i + 128 + TS2],
                                 start=False, stop=True, skip_group_check=True)
                with nc.vector.register() as rt, nc.vector.register() as rc:
                    nc.vector.reg_load([rt, rc], gint[i:i + 1, 0:2])
                    t0v = bass.make_scalar_value(rt, min_val=0, max_val=2)
                    csv = bass.make_scalar_value(rc, min_val=0, max_val=W - WS)
                    dslice = d4[:, bass.ds(t0v, 2), bass.ds(csv, WS)]
                    nc.vector.tensor_tensor(
                        out=dslice, in0=clb[:].rearrange("p (c w) -> p c w", c=2),
                        in1=dslice, op=OP.min)

        store_engs = [nc.sync, nc.gpsimd, nc.sync, nc.scalar]
        prev_last_tanh = None
        for m in range(2):
            sq = nc.scalar.activation(dmaps[m][:], dmaps[m][:], AF.Sqrt)
            if prev_last_tanh is not None:
                add_dep_helper(sq.ins, prev_last_tanh.ins, sync=False,
                               reason="act table order")
            if m == 0:
                # m0: fused tanh (stores overlap m1 strips anyway)
                th = nc.scalar.activation(dmaps[m][:], dmaps[m][:], AF.Tanh,
                                          scale=2.0)
                for t in range(NT):
                    seg = dmaps[m][:, t * W:(t + 1) * W]
                    store_engs[t].dma_start(y[m, t * 128:(t + 1) * 128, :], seg)
            else:
                # m1: per-tile tanh so stores pipeline into the tail
                for t in range(NT):
                    seg = dmaps[m][:, t * W:(t + 1) * W]
                    th = nc.scalar.activation(seg, seg, AF.Tanh, scale=2.0)
                    store_engs[t].dma_start(y[m, t * 128:(t + 1) * 128, :], seg)
            prev_last_tanh = th

    nc.compile()
    return nc


_CACHE = {}


def _get_built():
    if "nc" not in _CACHE:
        _CACHE["nc"] = _build()
        _CACHE["consts"] = _make_consts()
    return _CACHE["nc"], _CACHE["consts"]


def kernel(x: np.ndarray, coords: np.ndarray) -> np.ndarray:
    assert x.shape == (B, 3, H, W), x.shape
    assert coords.shape == (B, NPTS, 3), coords.shape
    coords = np.ascontiguousarray(coords, dtype=np.float32)

    nc, consts = _get_built()
    in_maps = [{"coords": coords[b], **consts} for b in range(B)]
    last_err = None
    for _attempt in range(3):
        try:
            res = run_bass_kernel_spmd(nc, in_maps, list(range(B)))
            break
        except Exception as e:  # device occasionally needs one recovery run
            last_err = e
    else:
        raise last_err
    out = np.stack([res.results[b]["y"] for b in range(B)])
    return out.astype(np.float32)

